# revision 1
# baseline (speedup 1.0000x reference)
"""Trainium2 Bass kernel for the CapsuleLayer routing problem.

Strategy (pure data-parallel over 8 NeuronCores, batch-sharded):
  u = x @ W  via a 3-term fp16 hi/lo split on the TensorEngine
  (xh@Wh + xh@Wl + xl@Wh, fp32 PSUM accumulate).  fp16 pairs carry
  ~22 mantissa bits, so accuracy matches fp32 (the routing softmax
  amplifies matmul error ~100-1000x: plain bf16/f32r fail the error
  gate, and native fp32 matmul runs at 1/4 rate).  Routing iterations
  restructured so squash() is a pure per-row scalar multiple
  (v = gamma * s), giving per 128-row tile: 4 broadcast-multiply
  passes (3 on GPSIMD, 1 on DVE) + 4 segmented reduces (DVE) over u,
  plus per-row softmax/gamma chains batched across 8-tile groups
  (DVE/ACT; one shared ln+exp ACT table set).

Layout: batch rows on partitions, features (16 caps x 32 dims) on the
free dim.  x is staged transposed+tiled from the host so each x-tile is
directly usable as the matmul stationary operand (lhsT).
"""

import sys
import os

for _p in ("/opt/trn_rl_repo", "/root/.axon_site/_ro/trn_rl_repo"):
    if os.path.isdir(_p) and _p not in sys.path:
        sys.path.insert(0, _p)
        break

import numpy as np
import ml_dtypes

import concourse.bass as bass
import concourse.bacc as bacc
import concourse.mybir as mybir
from concourse import tile
from concourse.bass_utils import run_bass_kernel_spmd

BF16 = mybir.dt.bfloat16
F16 = mybir.dt.float16
F32 = mybir.dt.float32
NP_BF16 = ml_dtypes.bfloat16

MM_DT_NAME = "float32"  # "float32" | "float32r"
NCORES = 8
B = 32768
K = 512
CAPS = 16
D = 32
ND = CAPS * D          # 512
BS = B // NCORES       # 4096 rows per core
P = 128                # partitions per tile
TILES = BS // P        # 32
G = 8                  # tiles per beta group
GROUPS = [(0, 2), (2, 6), (8, 8), (16, 8), (24, 8)]
NG = TILES // G        # 4
KCH = K // P           # 4 contraction chunks

AX = mybir.AxisListType.X
OP_ADD = mybir.AluOpType.add
OP_SUB = mybir.AluOpType.subtract
OP_MUL = mybir.AluOpType.mult
OP_MAX = mybir.AluOpType.max
FN = mybir.ActivationFunctionType
MM_DT = getattr(mybir.dt, MM_DT_NAME)


def _patch_act_tables():
    """Make the act-table-load pass resolve Exp and Ln to the combined
    natural_log_exp_and_others set so one table load serves the whole
    kernel (first-fit would otherwise alternate exp<->ln sets, ~2.7us per
    switch).  Indices must stay aligned with act_info.json, so only the
    function-membership sets are edited."""
    from concourse import hw_specs
    if getattr(hw_specs, "_capsule_patched", False):
        return
    orig = hw_specs.get_activation_tables

    def patched(module_arch):
        tables = {k: set(v) for k, v in orig(module_arch).items()}
        comb = "natural_log_exp_and_others"
        if comb in tables:
            for name, fns in tables.items():
                if name != comb:
                    fns.discard(FN.Exp)
                    fns.discard(FN.Ln)
        return tables

    import functools
    patched_cached = functools.cache(patched)
    hw_specs.get_activation_tables = patched_cached
    bacc.get_activation_tables = patched_cached
    hw_specs._capsule_patched = True


def _build_program():
    _patch_act_tables()
    nc = bacc.Bacc("TRN2", target_bir_lowering=False)

    xTh = nc.declare_dram_parameter("xh", [TILES, P, K], F16, isOutput=False)
    xTl = nc.declare_dram_parameter("xl", [TILES, P, K], F16, isOutput=False)
    WPK = 2 * (ND + D)  # 1088 packed weight cols per chunk
    Wpk = nc.declare_dram_parameter("Wpk", [KCH, P, WPK], F16, isOutput=False)
    vout = nc.declare_dram_parameter("v", [BS, D], F32, isOutput=True)
    vview = vout.ap().rearrange("(t p) d -> t p d", p=P)

    with tile.TileContext(nc) as tc:
        with (
            tc.tile_pool(name="wpool", bufs=1) as wpool,
            tc.tile_pool(name="xpool", bufs=4) as xpool,
            tc.tile_pool(name="upsum", bufs=3, space="PSUM") as upsum,
            tc.tile_pool(name="spsum", bufs=3, space="PSUM") as spsum,
            tc.tile_pool(name="upool", bufs=34) as upool,
            tc.tile_pool(name="tpool", bufs=10) as tpool,
            tc.tile_pool(name="s1pool", bufs=16) as s1pool,
            tc.tile_pool(name="gpool", bufs=4) as gpool,
            tc.tile_pool(name="spool", bufs=16) as spool,
        ):
            # --- constants: one DMA loads all fp16 hi/lo split weights ---
            wall = wpool.tile([P, KCH * WPK], F16, tag="wall")
            nc.sync.dma_start(
                wall[:].rearrange("p (c f) -> p c f", c=KCH),
                Wpk.ap().rearrange("c p f -> p c f"),
            )
            Wh = [wall[:, c * WPK: c * WPK + ND] for c in range(KCH)]
            Wl = [wall[:, c * WPK + ND: c * WPK + 2 * ND] for c in range(KCH)]
            Wsh = [wall[:, c * WPK + 2 * ND: c * WPK + 2 * ND + D]
                   for c in range(KCH)]
            Wsl = [wall[:, c * WPK + 2 * ND + D: c * WPK + 2 * ND + 2 * D]
                   for c in range(KCH)]

            for (T0, GS) in GROUPS:
                # ---- group buffers (fp32) ----
                q1g = gpool.tile([P, GS * CAPS], F32, tag="q1g")
                l2g = gpool.tile([P, GS * CAPS], F32, tag="l2g")
                e2g = gpool.tile([P, GS * CAPS], F32, tag="e2g")
                q2g = gpool.tile([P, GS * CAPS], F32, tag="q2g")
                l3g = gpool.tile([P, GS * CAPS], F32, tag="l3g")
                e3g = gpool.tile([P, GS * CAPS], F32, tag="e3g")
                s3g = gpool.tile([P, GS * D], F32, tag="s3g")
                sqg = gpool.tile([P, GS * D], F32, tag="sqg")
                vg = gpool.tile([P, GS * D], F32, tag="vg")
                nu1 = gpool.tile([P, GS], F32, tag="nu1")
                gam1 = gpool.tile([P, GS], F32, tag="gam1")
                m2 = gpool.tile([P, GS], F32, tag="m2")
                r2 = gpool.tile([P, GS], F32, tag="r2")
                sig2 = gpool.tile([P, GS], F32, tag="sig2")
                nu2 = gpool.tile([P, GS], F32, tag="nu2")
                del2 = gpool.tile([P, GS], F32, tag="del2")
                m3 = gpool.tile([P, GS], F32, tag="m3")
                r3 = gpool.tile([P, GS], F32, tag="r3")
                sig3 = gpool.tile([P, GS], F32, tag="sig3")
                nu3 = gpool.tile([P, GS], F32, tag="nu3")
                alp3 = gpool.tile([P, GS], F32, tag="alp3")
                tmpa = gpool.tile([P, GS], F32, tag="tmpa")
                tmpb = gpool.tile([P, GS], F32, tag="tmpb")
                prodg = gpool.tile([P, GS * CAPS], F32, tag="prodg")

                u_tiles = []
                # ---- two DMAs pull the whole group's split x tiles ----
                xgh = xpool.tile([P, GS * K], F16, tag="xgh")
                nc.sync.dma_start(
                    xgh[:].rearrange("p (t f) -> p t f", t=GS),
                    xTh[T0:T0 + GS].rearrange("t p f -> p t f"),
                )
                xgl = xpool.tile([P, GS * K], F16, tag="xgl")
                nc.sync.dma_start(
                    xgl[:].rearrange("p (t f) -> p t f", t=GS),
                    xTl[T0:T0 + GS].rearrange("t p f -> p t f"),
                )
                # ================= phase 1: matmul + q1 =================
                for t in range(GS):
                    u_ps = upsum.tile([P, ND], F32, tag="u_ps")
                    s_ps = spsum.tile([P, D], F32, tag="s_ps")
                    for c in range(KCH):
                        xh = xgh[:, t * K + c * P: t * K + (c + 1) * P]
                        xl = xgl[:, t * K + c * P: t * K + (c + 1) * P]
                        first = c == 0
                        last = c == KCH - 1
                        # u += xh@Wh + xh@Wl + xl@Wh   (fp16 hi/lo split)
                        nc.tensor.matmul(u_ps[:], xh, Wh[c],
                                         start=first, stop=False)
                        nc.tensor.matmul(u_ps[:], xh, Wl[c],
                                         start=False, stop=False)
                        nc.tensor.matmul(s_ps[:], xh, Wsh[c],
                                         start=first, stop=False)
                        nc.tensor.matmul(s_ps[:], xh, Wsl[c],
                                         start=False, stop=False)
                        nc.tensor.matmul(u_ps[:], xl, Wh[c],
                                         start=False, stop=last)
                        nc.tensor.matmul(s_ps[:], xl, Wsh[c],
                                         start=False, stop=last)

                    u_sb = upool.tile([P, ND], F32, tag="u_sb")
                    nc.scalar.copy(u_sb[:], u_ps[:])
                    s1_sb = s1pool.tile([P, D], F32, tag="s1_sb")
                    nc.scalar.copy(s1_sb[:], s_ps[:])
                    u_tiles.append(u_sb)

                    # t1 = u * bcast_k(s1)   [128, 16, 32]
                    t1 = tpool.tile([P, ND], F32, tag="tbuf")
                    u3 = u_sb[:].rearrange("p (k d) -> p k d", k=CAPS)
                    nc.gpsimd.tensor_tensor(
                        t1[:].rearrange("p (k d) -> p k d", k=CAPS),
                        u3,
                        s1_sb[:].unsqueeze(1).broadcast_to([P, CAPS, D]),
                        OP_MUL,
                    )
                    # q1 = sum_d t1
                    nc.vector.tensor_reduce(
                        q1g[:, t * CAPS:(t + 1) * CAPS],
                        t1[:].rearrange("p (k d) -> p k d", k=CAPS),
                        AX, OP_ADD,
                    )

                # ================= beta 1 =================
                q1v = q1g[:].rearrange("p (t k) -> p t k", t=GS)
                nc.vector.tensor_reduce(nu1[:], q1v, AX, OP_ADD)  # = 16*nu1
                # gamma1 = exp(0.5*ln(nu1)) / (1 + nu1);  ln(nu1) = ln(sum/16)
                nc.scalar.activation(tmpa[:], nu1[:], FN.Ln, scale=1.0 / CAPS)
                nc.scalar.activation(tmpa[:], tmpa[:], FN.Exp, scale=0.5)  # sqrt(nu1)
                nc.vector.tensor_scalar(tmpb[:], nu1[:], 1.0 / CAPS, 1.0, OP_MUL, OP_ADD)
                nc.vector.reciprocal(tmpb[:], tmpb[:])
                nc.vector.tensor_tensor(gam1[:], tmpa[:], tmpb[:], OP_MUL)
                # l2 = gamma1 * q1 ; m2 = max_k l2 ; e2 = exp(l2 - m2)
                g1b = gam1[:].unsqueeze(2).broadcast_to([P, GS, CAPS])
                nc.vector.tensor_tensor(
                    l2g[:].rearrange("p (t k) -> p t k", t=GS), q1v, g1b, OP_MUL)
                nc.vector.tensor_reduce(
                    m2[:], l2g[:].rearrange("p (t k) -> p t k", t=GS), AX, OP_MAX)
                nc.vector.tensor_tensor(
                    l2g[:].rearrange("p (t k) -> p t k", t=GS),
                    l2g[:].rearrange("p (t k) -> p t k", t=GS),
                    m2[:].unsqueeze(2).broadcast_to([P, GS, CAPS]),
                    OP_SUB,
                )
                nc.scalar.activation(e2g[:], l2g[:], FN.Exp)
                nc.vector.tensor_reduce(
                    r2[:], e2g[:].rearrange("p (t k) -> p t k", t=GS), AX, OP_ADD)
                nc.vector.reciprocal(r2[:], r2[:])

                # ================= phase 2: s2', q2' =================
                for t in range(GS):
                    u_sb = u_tiles[t]
                    u3 = u_sb[:].rearrange("p (k d) -> p k d", k=CAPS)
                    e2s = e2g[:, t * CAPS:(t + 1) * CAPS]
                    t2 = tpool.tile([P, ND], F32, tag="tbuf")
                    nc.gpsimd.tensor_tensor(
                        t2[:].rearrange("p (k d) -> p k d", k=CAPS),
                        u3,
                        e2s.unsqueeze(2).broadcast_to([P, CAPS, D]),
                        OP_MUL,
                    )
                    s2p = spool.tile([P, D], F32, tag="s2p")
                    nc.vector.tensor_reduce(
                        s2p[:],
                        t2[:].rearrange("p (k d) -> p d k", k=CAPS),
                        AX, OP_ADD,
                    )
                    t3 = tpool.tile([P, ND], F32, tag="tbuf")
                    nc.vector.tensor_tensor(
                        t3[:].rearrange("p (k d) -> p k d", k=CAPS),
                        u3,
                        s2p[:].unsqueeze(1).broadcast_to([P, CAPS, D]),
                        OP_MUL,
                    )
                    nc.vector.tensor_reduce(
                        q2g[:, t * CAPS:(t + 1) * CAPS],
                        t3[:].rearrange("p (k d) -> p k d", k=CAPS),
                        AX, OP_ADD,
                    )

                # ================= beta 2 =================
                q2v = q2g[:].rearrange("p (t k) -> p t k", t=GS)
                nc.vector.tensor_tensor(
                    prodg[:].rearrange("p (t k) -> p t k", t=GS),
                    e2g[:].rearrange("p (t k) -> p t k", t=GS), q2v, OP_MUL)
                nc.vector.tensor_reduce(
                    sig2[:], prodg[:].rearrange("p (t k) -> p t k", t=GS), AX, OP_ADD)
                nc.vector.tensor_tensor(tmpa[:], r2[:], r2[:], OP_MUL)
                nc.vector.tensor_tensor(nu2[:], sig2[:], tmpa[:], OP_MUL)
                nc.scalar.activation(tmpa[:], nu2[:], FN.Ln)
                nc.scalar.activation(tmpa[:], tmpa[:], FN.Exp, scale=0.5)
                nc.vector.tensor_scalar(tmpb[:], nu2[:], 1.0, 1.0, OP_MUL, OP_ADD)
                nc.vector.reciprocal(tmpb[:], tmpb[:])
                nc.vector.tensor_tensor(tmpa[:], tmpa[:], tmpb[:], OP_MUL)  # gamma2
                nc.vector.tensor_tensor(del2[:], tmpa[:], r2[:], OP_MUL)
                # l3 = l2 + del2 * q2'
                nc.vector.tensor_tensor(
                    prodg[:].rearrange("p (t k) -> p t k", t=GS),
                    q2v,
                    del2[:].unsqueeze(2).broadcast_to([P, GS, CAPS]),
                    OP_MUL,
                )
                nc.vector.tensor_tensor(l3g[:], l2g[:], prodg[:], OP_ADD)
                nc.vector.tensor_reduce(
                    m3[:], l3g[:].rearrange("p (t k) -> p t k", t=GS), AX, OP_MAX)
                nc.vector.tensor_tensor(
                    l3g[:].rearrange("p (t k) -> p t k", t=GS),
                    l3g[:].rearrange("p (t k) -> p t k", t=GS),
                    m3[:].unsqueeze(2).broadcast_to([P, GS, CAPS]),
                    OP_SUB,
                )
                nc.scalar.activation(e3g[:], l3g[:], FN.Exp)
                nc.vector.tensor_reduce(
                    r3[:], e3g[:].rearrange("p (t k) -> p t k", t=GS), AX, OP_ADD)
                nc.vector.reciprocal(r3[:], r3[:])

                # ================= phase 3: s3' =================
                for t in range(GS):
                    u_sb = u_tiles[t]
                    u3 = u_sb[:].rearrange("p (k d) -> p k d", k=CAPS)
                    e3s = e3g[:, t * CAPS:(t + 1) * CAPS]
                    t4 = tpool.tile([P, ND], F32, tag="tbuf")
                    nc.gpsimd.tensor_tensor(
                        t4[:].rearrange("p (k d) -> p k d", k=CAPS),
                        u3,
                        e3s.unsqueeze(2).broadcast_to([P, CAPS, D]),
                        OP_MUL,
                    )
                    nc.vector.tensor_reduce(
                        s3g[:, t * D:(t + 1) * D],
                        t4[:].rearrange("p (k d) -> p d k", k=CAPS),
                        AX, OP_ADD,
                    )

                # ================= beta 3 + output =================
                nc.vector.tensor_tensor(sqg[:], s3g[:], s3g[:], OP_MUL)
                nc.vector.tensor_reduce(
                    sig3[:], sqg[:].rearrange("p (t d) -> p t d", t=GS), AX, OP_ADD)
                nc.vector.tensor_tensor(tmpa[:], r3[:], r3[:], OP_MUL)
                nc.vector.tensor_tensor(nu3[:], sig3[:], tmpa[:], OP_MUL)
                nc.scalar.activation(tmpa[:], nu3[:], FN.Ln)
                nc.scalar.activation(tmpa[:], tmpa[:], FN.Exp, scale=0.5)
                nc.vector.tensor_scalar(tmpb[:], nu3[:], 1.0, 1.0, OP_MUL, OP_ADD)
                nc.vector.reciprocal(tmpb[:], tmpb[:])
                nc.vector.tensor_tensor(tmpa[:], tmpa[:], tmpb[:], OP_MUL)  # gamma3
                nc.vector.tensor_tensor(alp3[:], tmpa[:], r3[:], OP_MUL)
                nc.vector.tensor_tensor(
                    vg[:].rearrange("p (t d) -> p t d", t=GS),
                    s3g[:].rearrange("p (t d) -> p t d", t=GS),
                    alp3[:].unsqueeze(2).broadcast_to([P, GS, D]),
                    OP_MUL,
                )
                nc.sync.dma_start(
                    vview[T0:T0 + GS].rearrange("t p d -> p t d"),
                    vg[:].rearrange("p (t d) -> p t d", t=GS))

    nc.compile()
    return nc


_PROG_CACHE = {}


def _get_program():
    if "nc" not in _PROG_CACHE:
        _PROG_CACHE["nc"] = _build_program()
    return _PROG_CACHE["nc"]


def _split16(a):
    hi = a.astype(np.float16)
    lo = (a - hi.astype(np.float32)).astype(np.float16)
    return hi, lo


def _stage_inputs(x, W):
    x = np.ascontiguousarray(x, dtype=np.float32)
    W = np.ascontiguousarray(W, dtype=np.float32)
    Ws = W.reshape(K, CAPS, D).mean(axis=1, dtype=np.float32)
    Whh, Wll = _split16(W.reshape(KCH, P, ND))
    Wsh, Wsl = _split16(Ws.reshape(KCH, P, D))
    Wpk = np.ascontiguousarray(
        np.concatenate([Whh, Wll, Wsh, Wsl], axis=2))

    in_maps = []
    for core in range(NCORES):
        xs = x[core * BS:(core + 1) * BS]
        # lhsT tile layout: [tile, kappa_in_chunk(P), (chunk, j)]
        xt = np.ascontiguousarray(
            xs.reshape(TILES, P, KCH, P).transpose(0, 3, 2, 1)
        ).reshape(TILES, P, K)
        xh, xl = _split16(xt)
        in_maps.append({"xh": xh, "xl": xl, "Wpk": Wpk})
    return in_maps


def kernel(x, W, _trace=False, _trace_kwargs=None):
    nc = _get_program()
    in_maps = _stage_inputs(np.asarray(x), np.asarray(W))
    res = run_bass_kernel_spmd(
        nc, in_maps, list(range(NCORES)), trace=_trace,
        **(_trace_kwargs or {}),
    )
    out = np.concatenate(
        [np.asarray(res.results[i]["v"], dtype=np.float32) for i in range(NCORES)],
        axis=0,
    )
    if _trace:
        kernel._last_results = res
    return out

